# revision 12
# baseline (speedup 1.0000x reference)
"""Trainium2 (8-core) kernel for nn_NodeUpdateBlock: equivariant Linear +
FullyConnectedTensorProduct with 10 scalar (0e) one-hot attributes.

Self-contained: takes FULL inputs (as produced by the problem's
setup_inputs), distributes across the 8 NeuronCores internally, and
returns the FULL [N, 512] float32 output.

Strategy
--------
out_c = m_c @ (Wl_c * ls) + sum_a (att_a * f_c) @ (Wt_c[:,a,:] * ts)
per irrep-component plane c (1 scalar plane + 3 vector planes of 128
channels).  node_attrs rows are one-hot, so on the host we compute
z = argmax(attrs), sort nodes by z, split every attr-group evenly over
the 8 cores, and pad each per-core group to a multiple of 256 nodes.
The tensor product then needs exactly ONE matmul per (plane, group-run)
with a compile-time weight slice -- data-parallel over nodes, weights
replicated on every core.

The kernel is HBM-bandwidth bound, so all device I/O is fp16 (the
correctness gate is 2e-2 rel; fp16 quantization costs ~1e-3) and the
node tensors are pre-transposed to channel-major [512, S] on the host.
Channel-major tiles feed the PE directly as the moving operand (the
weights are stationary), so the kernel needs NO on-chip transposes:
per 512-node chunk and plane it is just 2-3 accumulating matmuls into
PSUM and one PSUM->SBUF fp16 copy.  A dense (non-one-hot) fallback
sums over all 10 attribute channels and stays correct for arbitrary
node_attrs.
"""

import math

import numpy as np

import concourse.bacc as bacc
import concourse.mybir as mybir
from concourse.tile import TileContext
from concourse.bass_utils import run_bass_kernel_spmd

MUL = 128
NA = 10
DIM = 512
N_CORES = 8
LIN_SCALE = 1.0 / math.sqrt(MUL)
TP_SCALE = 1.0 / math.sqrt(MUL * NA)
F32 = mybir.dt.float32
F32R = mybir.dt.float32r
F16 = mybir.dt.float16

LAST_RESULTS = None  # BassKernelResults of the most recent run (for testing)


def _col_perm():
    perm = list(range(MUL))
    for x in range(3):
        perm += [MUL + 3 * i + x for i in range(MUL)]
    return np.array(perm, dtype=np.int64)


COL_PERM = _col_perm()
COL_PERM_INV = np.argsort(COL_PERM)

CH = 512        # nodes per PSUM accumulation chunk (one bank: 512 fp32)
SLAB_CH = 4     # chunks per DMA slab (2048 nodes -> 4 KiB DMA lines)


def chunk_runs(tile_groups, tstart, ntiles):
    """Contiguous same-group runs (in nodes) for tiles [tstart, tstart+ntiles)."""
    runs = []
    s, cur = 0, tile_groups[tstart]
    for i in range(1, ntiles):
        g = tile_groups[tstart + i]
        if g != cur:
            runs.append((s * 128, (i - s) * 128, cur))
            s, cur = i, g
    runs.append((s * 128, (ntiles - s) * 128, cur))
    return runs


def build_program_t(n_tiles, tile_groups):
    """Channel-major fp16 program for one core (same program on all cores).

    Inputs:  mt [512,S] f16, ft [512,S] f16, wl [128,256] f16,
             wt [128,2560] f16       Output: out [512,S] f16
    All [512,S] tensors are channel-major: row r = permuted feature r,
    column n = node n; plane c = rows [c*128,(c+1)*128).

    Engine layout: sync issues wl/wt/mt loads + out stores (HWDGE),
    gpsimd issues ft loads (SWDGE); PSUM->SBUF fp16 copies rotate over
    scalar/vector/gpsimd.  Matmuls are batched per stationary weight so
    one LDWEIGHTS serves the three vector planes (keeps the PE stream
    dense -> HAM stays in the fast K=8/8 state).
    """
    S = n_tiles * 128
    SLAB = SLAB_CH * CH
    nc = bacc.Bacc("TRN2")
    mt = nc.dram_tensor("mt", [DIM, S], F16, kind="ExternalInput")
    ft = nc.dram_tensor("ft", [DIM, S], F16, kind="ExternalInput")
    wl = nc.dram_tensor("wl", [MUL, 2 * MUL], F16, kind="ExternalInput")
    wt = nc.dram_tensor("wt", [MUL, 2 * NA * MUL], F16, kind="ExternalInput")
    out = nc.dram_tensor("out", [DIM, S], F16, kind="ExternalOutput")

    slabs = []
    n0 = 0
    while n0 < S:
        slabs.append((n0, min(n0 + SLAB, S)))
        n0 = slabs[-1][1]

    # plane-major 3D views: [p=128, g=plane, n] so one DMA moves a whole slab
    mt4 = mt[:].rearrange("(g p) n -> p g n", g=4, p=128)
    ft4 = ft[:].rearrange("(g p) n -> p g n", g=4, p=128)
    out4 = out[:].rearrange("(g p) n -> p g n", g=4, p=128)

    with TileContext(nc) as tc:
        with (
            tc.tile_pool(name="const", bufs=1) as cpool,
            tc.tile_pool(name="io", bufs=3) as iopool,
            tc.tile_pool(name="ps", bufs=2, space="PSUM") as pspool,
        ):
            loaded = {}

            def load_slab(si, first=False):
                n0, n1 = slabs[si]
                W = n1 - n0
                mt_sb = iopool.tile([128, 4, SLAB], F16, tag="m_sb")
                ft_sb = iopool.tile([128, 4, SLAB], F16, tag="f_sb")
                if first:
                    # per-plane, plane 0 first on both queues, so the first
                    # chunk's matmuls can start as early as possible
                    for c in range(4):
                        nc.sync.dma_start(out=mt_sb[:, c, 0:W],
                                          in_=mt4[:, c, n0:n1])
                        if c == 0:
                            nc.sync.dma_start(out=wt_sb, in_=wt[:])
                        nc.gpsimd.dma_start(out=ft_sb[:, c, 0:W],
                                            in_=ft4[:, c, n0:n1])
                else:
                    nc.sync.dma_start(out=mt_sb[:, :, 0:W], in_=mt4[:, :, n0:n1])
                    nc.gpsimd.dma_start(out=ft_sb[:, :, 0:W], in_=ft4[:, :, n0:n1])
                loaded[si] = (mt_sb, ft_sb)

            wl_sb = cpool.tile([MUL, 2 * MUL], F16, tag="wl")
            nc.sync.dma_start(out=wl_sb, in_=wl[:])
            wt_sb = cpool.tile([MUL, 2 * NA * MUL], F16, tag="wt")
            load_slab(0, first=True)
            copy_engs = (nc.scalar, nc.vector)

            ncopy = 0
            for si, (n0, n1) in enumerate(slabs):
                W = n1 - n0
                while si + 2 < len(slabs) and si + 2 not in loaded and len(loaded) < 3:
                    load_slab(max(loaded) + 1)
                if si not in loaded:
                    load_slab(si)
                mt_sb, ft_sb = loaded.pop(si)

                o_sb = iopool.tile([128, 4, SLAB], F16, tag="o_sb")
                half = CH * max(1, (W // CH) // 2)
                for q in range(W // CH):
                    ck = (n0 // CH) + q
                    runs = chunk_runs(tile_groups, ck * 4, 4)
                    col = q * CH
                    y = [pspool.tile([128, CH], F32, tag=f"y{c}",
                                     name=f"y{c}") for c in range(4)]
                    # one LDWEIGHTS per stationary: wl0 -> y0; wl1 -> y1..y3
                    nc.tensor.matmul(y[0], wl_sb[:, 0:128],
                                     mt_sb[:, 0, col:col + CH],
                                     start=True, stop=False)
                    for c in (1, 2, 3):
                        nc.tensor.matmul(y[c], wl_sb[:, 128:256],
                                         mt_sb[:, c, col:col + CH],
                                         start=True, stop=False)
                    for ri, (off, size, g) in enumerate(runs):
                        last = ri == len(runs) - 1
                        nc.tensor.matmul(
                            y[0][:, off:off + size],
                            wt_sb[:, g * 128:(g + 1) * 128],
                            ft_sb[:, 0, col + off:col + off + size],
                            start=False, stop=last)
                    for ri, (off, size, g) in enumerate(runs):
                        last = ri == len(runs) - 1
                        for c in (1, 2, 3):
                            nc.tensor.matmul(
                                y[c][:, off:off + size],
                                wt_sb[:, (NA + g) * 128:(NA + g + 1) * 128],
                                ft_sb[:, c, col + off:col + off + size],
                                start=False, stop=last)
                    for c in range(4):
                        eng = copy_engs[ncopy % 2]
                        ncopy += 1
                        if eng is nc.scalar:
                            eng.copy(o_sb[:, c, col:col + CH], y[c])
                        else:
                            eng.tensor_copy(o_sb[:, c, col:col + CH], y[c])
                    # store each half-slab as soon as its copies are done
                    # (scalar queue: in-order after the copies feeding it)
                    kend = col + CH
                    if kend == half or kend == W:
                        k0 = 0 if kend == half else half
                        nc.scalar.dma_start(out=out4[:, :, n0 + k0:n0 + kend],
                                            in_=o_sb[:, :, k0:kend])

    nc.finalize()
    return nc


def build_program_dense(n_tiles, use_f32r=True):
    """Fallback for non-one-hot attrs: dense sum over the NA attr channels.

    Node-major fp32 layout.  Inputs: m [S,512], f [S,512], att [S,NA],
    wl [128,256], wt [128,2560], ident [128,128].  Output: out [S,512].
    """
    assert n_tiles % 4 == 0
    S = n_tiles * 128
    nc = bacc.Bacc("TRN2")
    m = nc.dram_tensor("m", [S, DIM], F32, kind="ExternalInput")
    f = nc.dram_tensor("f", [S, DIM], F32, kind="ExternalInput")
    att = nc.dram_tensor("att", [S, NA], F32, kind="ExternalInput")
    wl = nc.dram_tensor("wl", [MUL, 2 * MUL], F32, kind="ExternalInput")
    wt = nc.dram_tensor("wt", [MUL, 2 * NA * MUL], F32, kind="ExternalInput")
    ident = nc.dram_tensor("ident", [MUL, MUL], F32, kind="ExternalInput")
    out = nc.dram_tensor("out", [S, DIM], F32, kind="ExternalOutput")

    mm_dt = F32R if use_f32r else F32

    with TileContext(nc) as tc:
        with (
            tc.tile_pool(name="const", bufs=1) as cpool,
            tc.tile_pool(name="io", bufs=6) as iopool,
            tc.tile_pool(name="gsb", bufs=44) as gpool,
            tc.tile_pool(name="tmp", bufs=3) as tmpool,
            tc.tile_pool(name="tsb", bufs=12) as tpool,
            tc.tile_pool(name="ysb", bufs=6) as ypool,
            tc.tile_pool(name="psA", bufs=2, space="PSUM") as psA,
            tc.tile_pool(name="psB", bufs=2, space="PSUM") as psB,
            tc.tile_pool(name="psY", bufs=2, space="PSUM") as psY,
            tc.tile_pool(name="psO", bufs=2, space="PSUM") as psO,
        ):
            wl_sb = cpool.tile([MUL, 2 * MUL], F32, tag="wl")
            nc.sync.dma_start(out=wl_sb, in_=wl[:])
            wt_sb = cpool.tile([MUL, 2 * NA * MUL], F32, tag="wt")
            nc.sync.dma_start(out=wt_sb, in_=wt[:])
            id_sb = cpool.tile([MUL, MUL], F32, tag="ident")
            nc.sync.dma_start(out=id_sb, in_=ident[:])
            if use_f32r:
                wlr = cpool.tile([MUL, 2 * MUL], mm_dt, tag="wlr")
                nc.vector.tensor_copy(wlr, wl_sb)
                wtr = cpool.tile([MUL, 2 * NA * MUL], mm_dt, tag="wtr")
                nc.vector.tensor_copy(wtr, wt_sb)
                wl_sb, wt_sb = wlr, wtr

            for ck in range(n_tiles // 4):
                t0 = ck * 4
                m_tiles, f_tiles, a_tiles = [], [], []
                for nb in range(4):
                    mt = iopool.tile([128, DIM], F32, tag="m_sb")
                    nc.sync.dma_start(out=mt, in_=m[(t0 + nb) * 128:(t0 + nb + 1) * 128])
                    m_tiles.append(mt)
                    ft = iopool.tile([128, DIM], F32, tag="f_sb")
                    nc.sync.dma_start(out=ft, in_=f[(t0 + nb) * 128:(t0 + nb + 1) * 128])
                    f_tiles.append(ft)
                    at = iopool.tile([128, NA], F32, tag="a_sb")
                    nc.sync.dma_start(out=at, in_=att[(t0 + nb) * 128:(t0 + nb + 1) * 128])
                    a_tiles.append(at)

                # pre-scale: g[a][nb] = f[nb] * att[:, a]
                g_tiles = []
                for a in range(NA):
                    row = []
                    for nb in range(4):
                        gt = gpool.tile([128, DIM], F32, tag="g_sb")
                        nc.vector.tensor_scalar_mul(gt, f_tiles[nb], a_tiles[nb][:, a:a + 1])
                        row.append(gt)
                    g_tiles.append(row)

                y_sbs = []
                for c in range(4):
                    ct = 0 if c == 0 else 1
                    tm_ps = psA.tile([128, 512], F32, tag="tm_ps")
                    for nb in range(4):
                        nc.tensor.matmul(
                            tm_ps[:, nb * 128:(nb + 1) * 128],
                            m_tiles[nb][:, c * 128:(c + 1) * 128],
                            id_sb, is_transpose=True,
                        )
                    tm_sb = tmpool.tile([128, 512], mm_dt, tag="tm_sb")
                    nc.scalar.copy(tm_sb, tm_ps)

                    tg_sbs = []
                    for a in range(NA):
                        tg_ps = psB.tile([128, 512], F32, tag="tg_ps")
                        for nb in range(4):
                            nc.tensor.matmul(
                                tg_ps[:, nb * 128:(nb + 1) * 128],
                                g_tiles[a][nb][:, c * 128:(c + 1) * 128],
                                id_sb, is_transpose=True,
                            )
                        tg_sb = tpool.tile([128, 512], mm_dt, tag="tg_sb")
                        if a % 2 == 0:
                            nc.scalar.copy(tg_sb, tg_ps)
                        else:
                            nc.vector.tensor_copy(tg_sb, tg_ps)
                        tg_sbs.append(tg_sb)
                    y_ps = psY.tile([128, 512], F32, tag="y_ps")
                    nc.tensor.matmul(
                        y_ps,
                        wl_sb[:, ct * 128:(ct + 1) * 128],
                        tm_sb,
                        start=True, stop=False,
                    )
                    for a in range(NA):
                        nc.tensor.matmul(
                            y_ps,
                            wt_sb[:, (ct * NA + a) * 128:(ct * NA + a + 1) * 128],
                            tg_sbs[a],
                            start=False, stop=(a == NA - 1),
                        )
                    y_sb = ypool.tile([128, 512], F32, tag="y_sb")
                    if c % 2 == 0:
                        nc.scalar.copy(y_sb, y_ps)
                    else:
                        nc.vector.tensor_copy(y_sb, y_ps)
                    y_sbs.append(y_sb)

                for nb in range(4):
                    o_ps = psO.tile([128, 512], F32, tag="o_ps")
                    for c in range(4):
                        nc.tensor.matmul(
                            o_ps[:, c * 128:(c + 1) * 128],
                            y_sbs[c][:, nb * 128:(nb + 1) * 128],
                            id_sb, is_transpose=True,
                        )
                    o_sb = iopool.tile([128, DIM], F32, tag="o_sb")
                    if nb % 2 == 0:
                        nc.scalar.copy(o_sb, o_ps)
                    else:
                        nc.vector.tensor_copy(o_sb, o_ps)
                    nc.sync.dma_start(
                        out=out[(t0 + nb) * 128:(t0 + nb + 1) * 128], in_=o_sb
                    )

    nc.finalize()
    return nc


def pack_weights(Wl0, Wl1, Wt0, Wt1, dtype=np.float16):
    wl = np.concatenate([Wl0 * LIN_SCALE, Wl1 * LIN_SCALE], axis=1).astype(dtype)
    blocks = [Wt0[:, a, :] * TP_SCALE for a in range(NA)] + [
        Wt1[:, a, :] * TP_SCALE for a in range(NA)
    ]
    wt = np.concatenate(blocks, axis=1).astype(dtype)
    return np.ascontiguousarray(wl), np.ascontiguousarray(wt)


def plan_grouped(node_attrs):
    """One-hot grouping/sharding plan, or None if attrs are not one-hot."""
    N = node_attrs.shape[0]
    z = np.argmax(node_attrs, axis=1)
    onehot = np.zeros_like(node_attrs)
    onehot[np.arange(N), z] = 1.0
    if not np.array_equal(node_attrs, onehot):
        return None

    order = np.argsort(z, kind="stable")
    counts = np.bincount(z, minlength=NA)

    per_core_idx = [[] for _ in range(N_CORES)]
    per_core_gcnt = np.zeros((N_CORES, NA), dtype=np.int64)
    pos = 0
    for a in range(NA):
        ga = order[pos:pos + counts[a]]
        pos += counts[a]
        q, r = divmod(len(ga), N_CORES)
        off = 0
        for cidx in range(N_CORES):
            take = q + (1 if cidx < r else 0)
            per_core_idx[cidx].append(ga[off:off + take])
            per_core_gcnt[cidx, a] = take
            off += take

    gpad = np.maximum(
        128, (np.ceil(per_core_gcnt.max(axis=0) / 128) * 128).astype(np.int64)
    )
    S = int(gpad.sum())
    if S % 512 != 0:
        gpad[NA - 1] += 512 - (S % 512)
        S = int(gpad.sum())

    tile_groups = []
    for a in range(NA):
        tile_groups += [a] * (int(gpad[a]) // 128)

    plans = []
    goff = np.concatenate([[0], np.cumsum(gpad)])
    for cidx in range(N_CORES):
        idx = np.concatenate(per_core_idx[cidx])
        posn = np.concatenate(
            [goff[a] + np.arange(per_core_gcnt[cidx, a]) for a in range(NA)]
        ).astype(np.int64)
        plans.append((idx, posn))
    return dict(S=S, tile_groups=tile_groups, plans=plans)


_CACHE = {}


def kernel(m_i, node_feats, node_attrs, Wl0, Wl1, Wt0, Wt1):
    global LAST_RESULTS
    import os
    trace = bool(os.environ.get("KERNEL_TRACE"))
    m_i = np.ascontiguousarray(m_i, dtype=np.float32)
    node_feats = np.ascontiguousarray(node_feats, dtype=np.float32)
    node_attrs = np.ascontiguousarray(node_attrs, dtype=np.float32)
    N = m_i.shape[0]

    plan = plan_grouped(node_attrs)
    if plan is not None:
        wl, wt = pack_weights(Wl0, Wl1, Wt0, Wt1, dtype=np.float16)
        key = ("grouped_t", plan["S"], tuple(plan["tile_groups"]))
        if key not in _CACHE:
            _CACHE.clear()
            _CACHE[key] = build_program_t(plan["S"] // 128, plan["tile_groups"])
        nc = _CACHE[key]
        S = plan["S"]
        mp16 = m_i[:, COL_PERM].astype(np.float16)
        fp16 = node_feats[:, COL_PERM].astype(np.float16)
        in_maps = []
        for cidx in range(N_CORES):
            idx, posn = plan["plans"][cidx]
            mpad = np.zeros((S, DIM), dtype=np.float16)
            fpad = np.zeros((S, DIM), dtype=np.float16)
            mpad[posn] = mp16[idx]
            fpad[posn] = fp16[idx]
            in_maps.append(dict(mt=mpad.T.copy(order="C"),
                                ft=fpad.T.copy(order="C"), wl=wl, wt=wt))
        res = run_bass_kernel_spmd(
            nc, in_maps, core_ids=list(range(N_CORES)), trace=trace
        )
        LAST_RESULTS = res
        out = np.empty((N, DIM), dtype=np.float32)
        for cidx in range(N_CORES):
            idx, posn = plan["plans"][cidx]
            out[idx] = res.results[cidx]["out"].T[posn]
        return np.ascontiguousarray(out[:, COL_PERM_INV])

    # dense fallback (arbitrary attrs)
    wl, wt = pack_weights(Wl0, Wl1, Wt0, Wt1, dtype=np.float32)
    ident = np.eye(128, dtype=np.float32)
    mp = np.ascontiguousarray(m_i[:, COL_PERM])
    fp = np.ascontiguousarray(node_feats[:, COL_PERM])
    per_core = max(512, int(np.ceil(N / N_CORES / 512.0)) * 512)
    S = per_core
    key = ("dense", S)
    if key not in _CACHE:
        _CACHE.clear()
        _CACHE[key] = build_program_dense(S // 128)
    nc = _CACHE[key]
    in_maps = []
    bounds = []
    for cidx in range(N_CORES):
        lo = min(cidx * per_core, N)
        hi = min(lo + per_core, N)
        mpad = np.zeros((S, DIM), dtype=np.float32)
        fpad = np.zeros((S, DIM), dtype=np.float32)
        apad = np.zeros((S, NA), dtype=np.float32)
        mpad[:hi - lo] = mp[lo:hi]
        fpad[:hi - lo] = fp[lo:hi]
        apad[:hi - lo] = node_attrs[lo:hi]
        in_maps.append(dict(m=mpad, f=fpad, att=apad, wl=wl, wt=wt, ident=ident))
        bounds.append((lo, hi))
    res = run_bass_kernel_spmd(
        nc, in_maps, core_ids=list(range(N_CORES)), trace=trace
    )
    LAST_RESULTS = res
    out = np.empty((N, DIM), dtype=np.float32)
    for cidx, (lo, hi) in enumerate(bounds):
        out[lo:hi] = res.results[cidx]["out"][:hi - lo]
    return np.ascontiguousarray(out[:, COL_PERM_INV])
